# revision 16
# baseline (speedup 1.0000x reference)
"""DimeNet-style GNN forward on 8 Trainium2 NeuronCores (Bass/Tile).

Sharding: edges sorted by destination node j and split contiguously across 8
cores (node-aligned boundaries); triplets live on the core owning their
destination edge ji, sorted by ji so the triplet->edge scatter-add becomes
windowed one-hot matmuls accumulated in PSUM. Per layer each core computes
c = swish(lin_kj(m)) * (rbf @ W_rbf) on its shard; shards are AllGathered
(bf16) and each core indirect-DMA-gathers the rows its triplets reference
(the m[kj] message gather). Edge->graph readout skips the node tensor: edges
scatter straight into one persistent [HID, B] PSUM accumulator via batch
one-hot masks. Activations are kept transposed [HID=128 partitions, rows] in
bf16 with fp32 PSUM accumulation; small weights are replicated.
"""
import sys, os
for _p in ("/opt/trn_rl_repo", "/root/.axon_site/_ro/trn_rl_repo"):
    if os.path.isdir(_p) and _p not in sys.path:
        sys.path.insert(0, _p)

import math
import numpy as np
import jax.numpy as jnp

import concourse.bacc as bacc
import concourse.tile as tile
from concourse.tile import add_dep_helper
import concourse.mybir as mybir
from concourse.bass import IndirectOffsetOnAxis
from concourse.bass_utils import run_bass_kernel_spmd

DT = mybir.dt
OP = mybir.AluOpType
AF = mybir.ActivationFunctionType
AX = mybir.AxisListType

NC, P, HID, NRBF, NSBF, NBIL = 8, 128, 128, 16, 7, 8
NLAYERS, OUTBLK, B, CUTOFF = 4, 3, 128, 5.0
N, E, T = 50000, 200000, 200000
NP_ = 50048            # padded nodes (391*128)
EP = 25088             # padded per-core edges (196*128)
NWIN = EP // P         # 196
KT = 2                 # triplet tiles per window (fixed -> SPMD-safe)
TTILES = NWIN * KT     # 392
TSLOT = TTILES * P
EA_PAD = 200064
CFULL = NC * EP
CT = EP // 512         # 49

_cache = {}


def _bf16(a):
    return np.asarray(jnp.asarray(np.asarray(a, np.float32), dtype=jnp.bfloat16))


# --------------------------------------------------------------------------
# host-side preprocessing (index math + weight layout only)
# --------------------------------------------------------------------------
def _build_host(x, edge_index, edge_attr, batch, triplet_index, ph_enc,
                temp_enc, rec_flag, box_idx, params):
    g = np.asarray
    i_o = g(edge_index[0]).astype(np.int64)
    j_o = g(edge_index[1]).astype(np.int64)
    kj_o = g(triplet_index[0]).astype(np.int64)
    ji_o = g(triplet_index[1]).astype(np.int64)
    batch_np = g(batch).astype(np.int64)
    x_np = g(x).astype(np.float32)
    ea_np = g(edge_attr).astype(np.float32)

    order_e = np.argsort(j_o, kind="stable")
    j_s, i_s = j_o[order_e], i_o[order_e]
    inv_e = np.empty(E, np.int64)
    inv_e[order_e] = np.arange(E)

    e_bounds = [0]
    for p in range(1, NC):
        t = p * (E // NC)
        while t < E and j_s[t] == j_s[t - 1]:
            t += 1
        e_bounds.append(t)
    e_bounds.append(E)
    e_bounds = np.asarray(e_bounds, np.int64)
    assert (np.diff(e_bounds) <= EP).all()

    ji_n, kj_n = inv_e[ji_o], inv_e[kj_o]
    t_core = np.searchsorted(e_bounds[1:], ji_n, side="right")
    kj_core = np.searchsorted(e_bounds[1:], kj_n, side="right")
    kj_pad = (kj_core * EP + (kj_n - e_bounds[kj_core])).astype(np.int64)

    x_pad = np.zeros((NP_, 64), np.float32)
    x_pad[:N] = x_np
    ea_orig = np.zeros((EA_PAD, NRBF), np.float32)
    ea_orig[:E] = ea_np

    pr = params
    W = {}

    def lin(p_, name, bias=True):
        W[name + "_w"] = _bf16(g(p_["w"]))
        if bias:
            W[name + "_b"] = g(p_["b"]).astype(np.float32).reshape(-1, 1)

    lin(pr["emb"]["node"], "node")
    wl = g(pr["emb"]["lin"]["w"]).astype(np.float32)
    bl = g(pr["emb"]["lin"]["b"]).astype(np.float32)
    we = g(pr["emb"]["edge"]["w"]).astype(np.float32)
    be = g(pr["emb"]["edge"]["b"]).astype(np.float32)
    wr_ = g(pr["emb"]["rbf"]["w"]).astype(np.float32)
    br = g(pr["emb"]["rbf"]["b"]).astype(np.float32)
    W["emb1_w"] = _bf16(wl[0:128])
    W["embe_w"] = _bf16(we @ wl[128:256])
    W["embr_w"] = _bf16(wr_ @ wl[256:384])
    W["emb_b"] = (bl + be @ wl[128:256] + br @ wl[256:384]).reshape(-1, 1).astype(np.float32)

    sbf_all = []
    for li in range(NLAYERS):
        pl = pr["int"][li]
        lin(pl["lin_ji"], f"ji{li}")
        lin(pl["lin_kj"], f"kj{li}")
        W[f"rbf{li}_w"] = _bf16(g(pl["lin_rbf"]["w"]))
        sbf_all.append(g(pl["lin_sbf"]["w"]).astype(np.float32))
        W[f"W{li}"] = _bf16(g(pl["W"]).reshape(HID, NBIL * HID))
        for rn in ("res1", "res2"):
            lin(pl[rn]["lin1"], f"{rn}a{li}")
            lin(pl[rn]["lin2"], f"{rn}b{li}")
        lin(pl["lin_out"], f"lo{li}")
    W["sbf_all_w"] = _bf16(np.concatenate(sbf_all, axis=1))       # [7, 32]
    for oi in range(NLAYERS + 1):
        po = pr["out"][oi]
        for ri in range(OUTBLK):
            lin(po["res"][ri]["lin1"], f"ob{oi}r{ri}a")
            lin(po["res"][ri]["lin2"], f"ob{oi}r{ri}b")
        lin(po["lin"], f"ob{oi}l")
        W[f"ob{oi}l_brow"] = _bf16(g(po["lin"]["b"]).reshape(1, -1))

    c0w = g(pr["cond"][0]["w"]).astype(np.float32)
    W["c0_w"] = _bf16(c0w)                                         # [11, 64]
    W["c0_b"] = g(pr["cond"][0]["b"]).astype(np.float32).reshape(-1, 1)
    W["c1_w"] = _bf16(g(pr["cond"][1]["w"]))                       # [64, 32]
    W["c1_b"] = g(pr["cond"][1]["b"]).astype(np.float32).reshape(-1, 1)
    m0w = g(pr["mlp"][0]["w"]).astype(np.float32)                  # [160, 256]
    W["m0a_w"] = _bf16(m0w[0:128])
    W["m0b_w"] = _bf16(m0w[128:160])
    W["m0_b"] = g(pr["mlp"][0]["b"]).astype(np.float32).reshape(2, 128).T.copy()
    m1w = g(pr["mlp"][1]["w"]).astype(np.float32)                  # [256, 128]
    W["m1a_w"] = _bf16(m1w[0:128])
    W["m1b_w"] = _bf16(m1w[128:256])
    W["m1_b"] = g(pr["mlp"][1]["b"]).astype(np.float32).reshape(-1, 1)
    W["m2_w"] = _bf16(g(pr["mlp"][2]["w"]))                        # [128, 1]
    m2_b = float(g(pr["mlp"][2]["b"]).reshape(-1)[0])

    cont = np.stack([g(ph_enc), g(temp_enc), g(rec_flag)], -1).astype(np.float32)
    cond = np.concatenate([cont, g(pr["box_embed"])[g(box_idx).astype(np.int64)]], -1)
    W["condT"] = cond.T.copy().astype(np.float32)                  # [11, 128]

    consts = {
        "identity": np.eye(P, dtype=np.float32),
        "ident_bf": _bf16(np.eye(P)),
        "iota_bf": _bf16(np.tile(np.arange(P, dtype=np.float32)[None], (P, 1))),
        "freqs": np.tile((np.arange(1, NRBF + 1, dtype=np.float32)
                          * (math.pi / CUTOFF) / (2.0 * math.pi))[None], (P, 1)),
        "ones_bf": _bf16(np.ones((1, P))),
        "xT": x_pad.T.copy(),
        "xn_host": _bf16(x_pad @ np.asarray(pr["emb"]["node"]["w"], np.float32)
                         + np.asarray(pr["emb"]["node"]["b"], np.float32)),
        "ea_orig": ea_orig,
    }

    in_maps = []
    for p in range(NC):
        lo, hi = int(e_bounds[p]), int(e_bounds[p + 1])
        ne = hi - lo
        m = dict(W)
        m.update(consts)
        ii = np.zeros(EP, np.int32); ii[:ne] = i_s[lo:hi]
        jj = np.zeros(EP, np.int32); jj[:ne] = j_s[lo:hi]
        bj = np.full(EP, -1.0, np.float32)
        bj[:ne] = batch_np[j_s[lo:hi]].astype(np.float32)
        eaT = np.zeros((NRBF, EP), np.float32)
        eaT[:, :ne] = ea_np[order_e[lo:hi]].T
        ea_row = np.zeros((EP, NRBF), np.float32)
        ea_row[:ne] = ea_np[order_e[lo:hi]]
        m["i_idx"] = ii.reshape(NWIN, P).T.copy()
        m["j_idx"] = jj.reshape(NWIN, P).T.copy()
        m["bj_col"] = bj.reshape(NWIN, P).T.copy()
        m["eaT"] = eaT
        m["ea_row"] = ea_row

        sel = np.where(t_core == p)[0]
        jil = (ji_n[sel] - lo).astype(np.int64)
        o2 = np.argsort(jil, kind="stable")
        sel, jil = sel[o2], jil[o2]
        win = jil // P
        kj_slot = np.zeros(TSLOT, np.int32)
        ko_slot = np.zeros(TSLOT, np.int32)
        jo_slot = np.zeros(TSLOT, np.int32)
        jirel = np.full(TSLOT, -1.0, np.float32)
        st = np.searchsorted(win, np.arange(NWIN))
        en = np.searchsorted(win, np.arange(NWIN), side="right")
        for w in range(NWIN):
            s, e_ = int(st[w]), int(en[w])
            cnt = e_ - s
            assert cnt <= KT * P, f"window overflow {cnt}"
            base = w * KT * P
            kj_slot[base:base + cnt] = kj_pad[sel[s:e_]]
            ko_slot[base:base + cnt] = kj_o[sel[s:e_]]
            jo_slot[base:base + cnt] = ji_o[sel[s:e_]]
            jirel[base:base + cnt] = (jil[s:e_] - w * P).astype(np.float32)
        m["kj_idx"] = kj_slot.reshape(TTILES, P).T.copy()
        m["ko_idx"] = ko_slot.reshape(TTILES, P).T.copy()
        m["jo_idx"] = jo_slot.reshape(TTILES, P).T.copy()
        m["jirel"] = jirel.reshape(TTILES, P).T.copy()
        in_maps.append(m)
    return in_maps, m2_b


# --------------------------------------------------------------------------
# device program
# --------------------------------------------------------------------------
def _build_program(map0, m2_b):
    nc = bacc.Bacc("TRN2", target_bir_lowering=False, debug=False, num_devices=NC)
    aps = {}
    for name, arr in map0.items():
        if arr.dtype == np.float32:
            dt = DT.float32
        elif arr.dtype == np.int32:
            dt = DT.int32
        else:
            dt = DT.bfloat16
        aps[name] = nc.dram_tensor(name, list(arr.shape), dt, kind="ExternalInput").ap()
    out_ap = nc.dram_tensor("out", [1, B], DT.float32, kind="ExternalOutput").ap()
    DBG = os.environ.get("KDBG") == "1"
    dbg = {}
    if DBG:
        for nm, shp, dt_ in [("d_xn", [512, HID], DT.bfloat16),
                             ("d_m0", [P, 512], DT.bfloat16),
                             ("d_c", [512, HID], DT.bfloat16),
                             ("d_xji", [P, 512], DT.bfloat16),
                             ("d_agg", [P, 512], DT.bfloat16),
                             ("d_m1", [P, 512], DT.bfloat16),
                             ("d_hg", [P, B], DT.float32),
                             ("d_sbfp", [P, 64], DT.float32),
                             ("d_rbf", [NRBF, 512], DT.bfloat16)]:
            dbg[nm] = nc.dram_tensor(nm, shp, dt_, kind="ExternalOutput").ap()

    BF, F32 = DT.bfloat16, DT.float32
    with tile.TileContext(nc) as tc:
        from contextlib import ExitStack
        ctx = ExitStack()
        sb = ctx.enter_context(tc.tile_pool(name="sb", bufs=3))
        sbw = ctx.enter_context(tc.tile_pool(name="sbw", bufs=1))
        ps = ctx.enter_context(tc.tile_pool(name="ps", bufs=2, space="PSUM"))
        pst = ctx.enter_context(tc.tile_pool(name="pst", bufs=2, space="PSUM"))
        psG = ctx.enter_context(tc.tile_pool(name="psG", bufs=1, space="PSUM"))
        psh = ctx.enter_context(tc.tile_pool(name="psh", bufs=1, space="PSUM"))
        dr = ctx.enter_context(tc.tile_pool(name="dr", bufs=1, space="DRAM"))

        wt = {}
        for name, a in aps.items():
            if name in ("xT", "eaT", "ea_row", "ea_orig", "i_idx", "j_idx",
                        "kj_idx", "ko_idx", "jo_idx", "xn_host"):
                continue
            t = sbw.tile(list(a.shape), a.dtype, tag=name, name="w_" + name)
            nc.sync.dma_start(out=t[:], in_=a[:])
            wt[name] = t
        ident, ident_bf = wt["identity"], wt["ident_bf"]
        iota, freqs = wt["iota_bf"], wt["freqs"]

        idx_sb = {}
        for name in ("i_idx", "j_idx", "kj_idx", "ko_idx", "jo_idx"):
            a = aps[name]
            t = sbw.tile(list(a.shape), a.dtype, tag=name, name="i_" + name)
            nc.sync.dma_start(out=t[:], in_=a[:])
            idx_sb[name] = t

        rbfT_d = dr.tile([NRBF, EP], BF, tag="rbfT_d", name="rbfT_d")
        sbfp = sbw.tile([P, TTILES * 32], F32, tag="sbfp")

        # DRAM scratch
        xn_row = dr.tile([NP_, HID], BF, tag="xn_row")
        mT = [dr.tile([P, EP], BF, tag=f"mT{l}", name=f"mT{l}") for l in range(NLAYERS + 1)]
        xjiT = dr.tile([P, EP], BF, tag="xjiT")
        aggT = dr.tile([P, EP], BF, tag="aggT")
        c_shard = dr.tile([EP, HID], BF, tag="c_shard")
        c_full = dr.tile([CFULL, HID], BF, tag="c_full")
        hg_d = dr.tile([P, B], F32, tag="hg_d")
        hg_r = dr.tile([P, B], F32, tag="hg_r")

        hg_ps = psh.tile([P, B], F32, space="PSUM")
        hg_n = [0]
        HG_TOTAL = (NLAYERS + 1) * NWIN

        def gather(table, idx_col, width, dtype, after=None):
            tl = sb.tile([P, width], dtype, tag="gather", name="gtile")
            gi = nc.gpsimd.indirect_dma_start(
                out=tl[:], out_offset=None, in_=table,
                in_offset=IndirectOffsetOnAxis(ap=idx_col, axis=0))
            if after is not None:
                add_dep_helper(gi.ins, after, True, "indirect gather after table write")
            return tl

        def silu_evict(psum, bias, tag):
            o = sb.tile([P, psum.shape[-1]], BF, tag=tag, name="se_" + tag)
            nc.scalar.activation(out=o[:], in_=psum[:], func=AF.Silu,
                                 bias=bias, scale=1.0)
            return o

        def residual(xT, wa, ba, wb, bb, width, pref):
            s0 = sb.tile([P, width], BF, tag=pref + "s0", name=pref + "s0")
            nc.scalar.activation(out=s0[:], in_=xT[:], func=AF.Silu, scale=1.0)
            p1 = ps.tile([P, width], F32, space="PSUM", tag="pA", name="p1")
            nc.tensor.matmul(out=p1[:], lhsT=wt[wa][:], rhs=s0[:], start=True, stop=True)
            s1 = silu_evict(p1, wt[ba][:, :1], pref + "s1")
            p2 = ps.tile([P, width], F32, space="PSUM", tag="pA", name="p2")
            nc.tensor.matmul(out=p2[:], lhsT=wt[wb][:], rhs=s1[:], start=True, stop=True)
            u = sb.tile([P, width], BF, tag=pref + "u", name=pref + "u")
            nc.vector.tensor_scalar_add(out=u[:], in0=p2[:], scalar1=wt[bb][:, :1])
            r = sb.tile([P, width], BF, tag=pref + "r", name=pref + "r")
            nc.vector.tensor_tensor(out=r[:], in0=u[:], in1=xT[:], op=OP.add)
            return r

        # ---------------- Phase B: xn table ------------------------------
        bn_ps = pst.tile([P, P], F32, space="PSUM", tag="t1")
        nc.tensor.transpose(out=bn_ps[:], in_=wt["node_b"][:, :1].to_broadcast([P, P]),
                            identity=ident[:])
        bn_bc = sbw.tile([P, P], F32, tag="bn_bc")
        nc.vector.tensor_copy(out=bn_bc[:], in_=bn_ps[:])
        xn_writes = []
        for nb in range(NP_ // 512):
            xt = sb.tile([64, 512], F32, tag="xt")
            nc.sync.dma_start(out=xt[:], in_=aps["xT"][:, nb * 512:(nb + 1) * 512])
            xtb = sb.tile([64, 512], BF, tag="xtb")
            nc.vector.tensor_copy(out=xtb[:], in_=xt[:])
            stage = sb.tile([P, 512], BF, tag="xn_st")
            for k in range(4):
                pp = pst.tile([P, P], F32, space="PSUM", tag="t1")
                nc.tensor.matmul(out=pp[:], lhsT=xtb[:, k * P:(k + 1) * P],
                                 rhs=wt["node_w"][:], start=True, stop=True)
                nc.vector.tensor_tensor(out=stage[:, k * P:(k + 1) * P], in0=pp[:],
                                        in1=bn_bc[:], op=OP.add)
            dst = xn_row[nb * 512:(nb + 1) * 512, :].rearrange("(k p) f -> p k f", k=4, p=P)
            nc.sync.dma_start(out=dst, in_=stage[:])

        join_t = sbw.tile([1, 1], F32, tag="join_t", name="join_t")
        ji_ = nc.vector.tensor_scalar_add(out=join_t[:], in0=join_t[:], scalar1=0.0)
        for w in xn_writes:
            add_dep_helper(ji_.ins, w, True, "join waits for all xn writes")
        xn_join = ji_.ins

        # ---------------- Phase C0: rbf ----------------------------------
        for t in range(NWIN):
            ea = sb.tile([P, NRBF], F32, tag="ea")
            nc.sync.dma_start(out=ea[:], in_=aps["ea_row"][t * P:(t + 1) * P, :])
            sq = sb.tile([P, NRBF], F32, tag="sq")
            nc.vector.tensor_tensor(out=sq[:], in0=ea[:], in1=ea[:], op=OP.mult)
            d2 = sb.tile([P, 1], F32, tag="d2")
            nc.vector.reduce_sum(out=d2[:], in_=sq[:], axis=AX.X)
            d = sb.tile([P, 1], F32, tag="d")
            nc.scalar.activation(out=d[:], in_=d2[:], func=AF.Sqrt, scale=1.0)
            dp = sb.tile([P, 1], F32, tag="dp")
            nc.vector.tensor_scalar_add(out=dp[:], in0=d[:], scalar1=1e-8)
            rec = sb.tile([P, 1], F32, tag="rec")
            nc.vector.reciprocal(out=rec[:], in_=dp[:])
            # q = f*d/(2pi); y = q - round(q); sin(2pi*y) == sin(f*d)
            q_ = sb.tile([P, NRBF], F32, tag="q_")
            nc.vector.tensor_scalar_mul(out=q_[:], in0=freqs[:], scalar1=d[:, :1])
            qi = sb.tile([P, NRBF], DT.int32, tag="qi")
            nc.vector.tensor_copy(out=qi[:], in_=q_[:])
            qf = sb.tile([P, NRBF], F32, tag="qf")
            nc.vector.tensor_copy(out=qf[:], in_=qi[:])
            yr = sb.tile([P, NRBF], F32, tag="yr")
            nc.vector.tensor_tensor(out=yr[:], in0=q_[:], in1=qf[:], op=OP.subtract)
            sins = sb.tile([P, NRBF], F32, tag="sins")
            nc.scalar.activation(out=sins[:], in_=yr[:], func=AF.Sin,
                                 scale=2.0 * math.pi)
            rrow = sb.tile([P, NRBF], F32, tag="rrow")
            nc.vector.tensor_scalar_mul(out=rrow[:], in0=sins[:], scalar1=rec[:, :1])
            tp = pst.tile([NRBF, P], F32, space="PSUM", tag="t1")
            nc.tensor.transpose(out=tp[:], in_=rrow[:], identity=ident[:])
            rbs = sb.tile([NRBF, P], BF, tag="rbs", name="rbs")
            nc.vector.tensor_copy(out=rbs[:], in_=tp[:])
            nc.sync.dma_start(out=rbfT_d[:, t * P:(t + 1) * P], in_=rbs[:])

        # ---------------- Phase C2: sbf / sbf_proj ------------------------
        for t in range(TTILES):
            ekj = gather(aps["ea_orig"][:], idx_sb["ko_idx"][:, t:t + 1], NRBF, F32)
            eji = gather(aps["ea_orig"][:], idx_sb["jo_idx"][:, t:t + 1], NRBF, F32)
            ab_ = sb.tile([P, NRBF], F32, tag="ab_")
            nc.vector.tensor_tensor(out=ab_[:], in0=ekj[:], in1=eji[:], op=OP.mult)
            ab = sb.tile([P, 1], F32, tag="ab")
            nc.vector.reduce_sum(out=ab[:], in_=ab_[:], axis=AX.X)
            aa_ = sb.tile([P, NRBF], F32, tag="aa_")
            nc.vector.tensor_tensor(out=aa_[:], in0=ekj[:], in1=ekj[:], op=OP.mult)
            aa = sb.tile([P, 1], F32, tag="aa")
            nc.vector.reduce_sum(out=aa[:], in_=aa_[:], axis=AX.X)
            bb_ = sb.tile([P, NRBF], F32, tag="bb_")
            nc.vector.tensor_tensor(out=bb_[:], in0=eji[:], in1=eji[:], op=OP.mult)
            bb = sb.tile([P, 1], F32, tag="bb")
            nc.vector.reduce_sum(out=bb[:], in_=bb_[:], axis=AX.X)
            q = sb.tile([P, 1], F32, tag="q")
            nc.vector.tensor_tensor(out=q[:], in0=aa[:], in1=bb[:], op=OP.mult)
            sq_ = sb.tile([P, 1], F32, tag="sq_")
            nc.scalar.activation(out=sq_[:], in_=q[:], func=AF.Sqrt, scale=1.0)
            sqc = sb.tile([P, 1], F32, tag="sqc")
            nc.vector.tensor_scalar_max(out=sqc[:], in0=sq_[:], scalar1=1e-16)
            rc = sb.tile([P, 1], F32, tag="rc")
            nc.vector.reciprocal(out=rc[:], in_=sqc[:])
            cs = sb.tile([P, 1], F32, tag="cs")
            nc.vector.tensor_scalar(out=cs[:], in0=ab[:], scalar1=rc[:, :1],
                                    scalar2=0.999, op0=OP.mult, op1=OP.min)
            cosv = sb.tile([P, 1], F32, tag="cosv")
            nc.vector.tensor_scalar_max(out=cosv[:], in0=cs[:], scalar1=-0.999)
            c2 = sb.tile([P, 1], F32, tag="c2")
            nc.vector.tensor_tensor(out=c2[:], in0=cosv[:], in1=cosv[:], op=OP.mult)
            sbf_t = sb.tile([P, NSBF], F32, tag="sbf_t")
            nc.scalar.activation(out=sbf_t[:, 0:1], in_=c2[:], func=AF.Sqrt,
                                 scale=-1.0, bias=1.0)
            twoc = sb.tile([P, 1], F32, tag="twoc")
            nc.vector.tensor_scalar_mul(out=twoc[:], in0=cosv[:], scalar1=2.0)
            nc.vector.tensor_scalar(out=sbf_t[:, 1:2], in0=sbf_t[:, 0:1],
                                    scalar1=twoc[:, :1], scalar2=None, op0=OP.mult)
            for k in range(2, NSBF):
                tmp = sb.tile([P, 1], F32, tag="sbtmp")
                nc.vector.tensor_scalar(out=tmp[:], in0=sbf_t[:, k - 1:k],
                                        scalar1=twoc[:, :1], scalar2=None, op0=OP.mult)
                nc.vector.tensor_tensor(out=sbf_t[:, k:k + 1], in0=tmp[:],
                                        in1=sbf_t[:, k - 2:k - 1], op=OP.subtract)
            tp7 = pst.tile([NSBF, P], F32, space="PSUM", tag="t1")
            nc.tensor.transpose(out=tp7[:], in_=sbf_t[:], identity=ident[:])
            sbfTt = sb.tile([NSBF, P], BF, tag="sbfTt")
            nc.vector.tensor_copy(out=sbfTt[:], in_=tp7[:])
            pj = pst.tile([P, 32], F32, space="PSUM", tag="t1")
            nc.tensor.matmul(out=pj[:], lhsT=sbfTt[:], rhs=wt["sbf_all_w"][:],
                             start=True, stop=True)
            nc.vector.tensor_copy(out=sbfp[:, t * 32:(t + 1) * 32], in_=pj[:])

        # ---------------- Phase C: embedding -> mT[0] ---------------------
        xn_src = aps["xn_host"][:] if os.environ.get("KHOSTXN") == "1" else xn_row[:]
        for t in range(NWIN):
            xi = gather(xn_src, idx_sb["i_idx"][:, t:t + 1], HID, BF, after=xn_join)
            xj = gather(xn_src, idx_sb["j_idx"][:, t:t + 1], HID, BF, after=xn_join)
            xij = sb.tile([P, HID], BF, tag="xij")
            nc.vector.tensor_tensor(out=xij[:], in0=xi[:], in1=xj[:], op=OP.mult)
            tp = pst.tile([P, P], BF, space="PSUM", tag="t1", name="tp")
            nc.tensor.transpose(out=tp[:], in_=xij[:], identity=ident_bf[:])
            xijT = sb.tile([P, P], BF, tag="xijT")
            nc.vector.tensor_copy(out=xijT[:], in_=tp[:])
            ea = sb.tile([NRBF, P], F32, tag="eaTl")
            nc.sync.dma_start(out=ea[:], in_=aps["eaT"][:, t * P:(t + 1) * P])
            eab = sb.tile([NRBF, P], BF, tag="eaTb")
            nc.vector.tensor_copy(out=eab[:], in_=ea[:])
            pm = ps.tile([P, P], F32, space="PSUM", tag="pA")
            nc.tensor.matmul(out=pm[:], lhsT=wt["emb1_w"][:], rhs=xijT[:],
                             start=True, stop=False)
            nc.tensor.matmul(out=pm[:], lhsT=wt["embe_w"][:], rhs=eab[:],
                             start=False, stop=False)
            rbl = sb.tile([NRBF, P], BF, tag="rbl", name="rbl")
            nc.sync.dma_start(out=rbl[:], in_=rbfT_d[:, t * P:(t + 1) * P])
            nc.tensor.matmul(out=pm[:], lhsT=wt["embr_w"][:],
                             rhs=rbl[:], start=False, stop=True)
            m0t = silu_evict(pm, wt["emb_b"][:, :1], "m0t")
            nc.sync.dma_start(out=mT[0][:, t * P:(t + 1) * P], in_=m0t[:])

        def hg_scatter(me_row, eb):
            sg = sb.tile([P, B], BF, tag="sg")
            nc.vector.tensor_scalar(out=sg[:], in0=iota[:],
                                    scalar1=wt["bj_col"][:, eb:eb + 1],
                                    scalar2=None, op0=OP.is_equal)
            nc.tensor.matmul(out=hg_ps[:], lhsT=me_row[:], rhs=sg[:],
                             start=(hg_n[0] == 0), stop=(hg_n[0] == HG_TOTAL * 4 - 1),
                             skip_group_check=True)
            hg_n[0] += 1

        def out_block(oi, mcur):
            for c in range(CT):
                mt = sb.tile([P, 512], BF, tag="ob_m")
                nc.sync.dma_start(out=mt[:], in_=mcur[:, c * 512:(c + 1) * 512])
                r = mt
                for ri in range(OUTBLK):
                    r = residual(r, f"ob{oi}r{ri}a_w", f"ob{oi}r{ri}a_b",
                                 f"ob{oi}r{ri}b_w", f"ob{oi}r{ri}b_b", 512, "ob")
                for k in range(4):
                    pr_ = pst.tile([P, P], F32, space="PSUM", tag="t1")
                    nc.tensor.matmul(out=pr_[:], lhsT=r[:, k * P:(k + 1) * P],
                                     rhs=wt[f"ob{oi}l_w"][:], start=True, stop=False)
                    nc.tensor.matmul(out=pr_[:], lhsT=wt["ones_bf"][:1, :],
                                     rhs=wt[f"ob{oi}l_brow"][:1, :],
                                     start=False, stop=True)
                    me = sb.tile([P, P], BF, tag="ob_me")
                    nc.scalar.activation(out=me[:], in_=pr_[:], func=AF.Silu, scale=1.0)
                    hg_scatter(me, c * 4 + k)

        out_block(0, mT[0])

        # ---------------- layers ------------------------------------------
        for l in range(NLAYERS):
            # P1: x_ji, c
            for c in range(CT):
                sl = slice(c * 512, (c + 1) * 512)
                mt = sb.tile([P, 512], BF, tag="p1_m")
                nc.sync.dma_start(out=mt[:], in_=mT[l][:, sl])
                p1 = ps.tile([P, 512], F32, space="PSUM", tag="pA")
                nc.tensor.matmul(out=p1[:], lhsT=wt[f"ji{l}_w"][:], rhs=mt[:],
                                 start=True, stop=True)
                xji = silu_evict(p1, wt[f"ji{l}_b"][:, :1], "p1_xji")
                nc.sync.dma_start(out=xjiT[:, sl], in_=xji[:])
                p2 = ps.tile([P, 512], F32, space="PSUM", tag="pA")
                nc.tensor.matmul(out=p2[:], lhsT=wt[f"kj{l}_w"][:], rhs=mt[:],
                                 start=True, stop=True)
                t1 = silu_evict(p2, wt[f"kj{l}_b"][:, :1], "p1_t1")
                rbl5 = sb.tile([NRBF, 512], BF, tag="rbl5", name="rbl5")
                nc.sync.dma_start(out=rbl5[:], in_=rbfT_d[:, sl])
                p3 = ps.tile([P, 512], F32, space="PSUM", tag="pA")
                nc.tensor.matmul(out=p3[:], lhsT=wt[f"rbf{l}_w"][:],
                                 rhs=rbl5[:], start=True, stop=True)
                ct_ = sb.tile([P, 512], BF, tag="p1_c")
                nc.vector.tensor_tensor(out=ct_[:], in0=t1[:], in1=p3[:], op=OP.mult)
                stage = sb.tile([P, 512], BF, tag="p1_st")
                for k in range(4):
                    tp = pst.tile([P, P], BF, space="PSUM", tag="t1", name="tp")
                    nc.tensor.transpose(out=tp[:], in_=ct_[:, k * P:(k + 1) * P],
                                        identity=ident_bf[:])
                    nc.vector.tensor_copy(out=stage[:, k * P:(k + 1) * P], in_=tp[:])
                dst = c_shard[c * 512:(c + 1) * 512, :].rearrange(
                    "(k p) f -> p k f", k=4, p=P)
                nc.sync.dma_start(out=dst, in_=stage[:])

            ag_inst = nc.gpsimd.collective_compute(
                "AllGather", OP.bypass, replica_groups=[list(range(NC))],
                ins=[c_shard[:]], outs=[c_full[:]]).ins

            # P3: triplets -> aggT
            for w in range(NWIN):
                G = psG.tile([P, NBIL * P], F32, space="PSUM", tag="G")
                for kt in range(KT):
                    t = w * KT + kt
                    cg = gather(c_full[:], idx_sb["kj_idx"][:, t:t + 1], HID, BF,
                                after=ag_inst)
                    for b in range(NBIL):
                        S = sb.tile([P, P], BF, tag="S")
                        nc.vector.tensor_scalar(
                            out=S[:], in0=iota[:],
                            scalar1=wt["jirel"][:, t:t + 1],
                            scalar2=sbfp[:, t * 32 + l * 8 + b:t * 32 + l * 8 + b + 1],
                            op0=OP.is_equal, op1=OP.mult)
                        nc.tensor.matmul(out=G[:, b * P:(b + 1) * P], lhsT=cg[:],
                                         rhs=S[:], start=(kt == 0), stop=(kt == KT - 1))
                Gs = sb.tile([P, NBIL * P], BF, tag="Gs")
                for b in range(NBIL):
                    slb = slice(b * P, (b + 1) * P)
                    if b % 2 == 0:
                        nc.vector.tensor_copy(out=Gs[:, slb], in_=G[:, slb])
                    else:
                        nc.scalar.activation(out=Gs[:, slb], in_=G[:, slb],
                                             func=AF.Copy, scale=1.0)
                pa = pst.tile([P, P], F32, space="PSUM", tag="t1")
                for b in range(NBIL):
                    slb = slice(b * P, (b + 1) * P)
                    nc.tensor.matmul(out=pa[:], lhsT=wt[f"W{l}"][:, slb],
                                     rhs=Gs[:, slb], start=(b == 0), stop=(b == NBIL - 1))
                av = sb.tile([P, P], BF, tag="av")
                nc.vector.tensor_copy(out=av[:], in_=pa[:])
                nc.sync.dma_start(out=aggT[:, w * P:(w + 1) * P], in_=av[:])

            # P4: residual chain -> mT[l+1]
            for c in range(CT):
                sl = slice(c * 512, (c + 1) * 512)
                mt = sb.tile([P, 512], BF, tag="p4_m")
                nc.sync.dma_start(out=mt[:], in_=mT[l][:, sl])
                xji = sb.tile([P, 512], BF, tag="p4_x")
                nc.sync.dma_start(out=xji[:], in_=xjiT[:, sl])
                ag = sb.tile([P, 512], BF, tag="p4_a")
                nc.sync.dma_start(out=ag[:], in_=aggT[:, sl])
                r0 = sb.tile([P, 512], BF, tag="p4_r0")
                nc.vector.tensor_tensor(out=r0[:], in0=xji[:], in1=ag[:], op=OP.add)
                r1 = residual(r0, f"res1a{l}_w", f"res1a{l}_b",
                              f"res1b{l}_w", f"res1b{l}_b", 512, "p4")
                r2 = residual(r1, f"res2a{l}_w", f"res2a{l}_b",
                              f"res2b{l}_w", f"res2b{l}_b", 512, "p4")
                po = ps.tile([P, 512], F32, space="PSUM", tag="pA")
                nc.tensor.matmul(out=po[:], lhsT=wt[f"lo{l}_w"][:], rhs=r2[:],
                                 start=True, stop=True)
                sw = silu_evict(po, wt[f"lo{l}_b"][:, :1], "p4_sw")
                mn = sb.tile([P, 512], BF, tag="p4_mn")
                nc.vector.tensor_tensor(out=mn[:], in0=sw[:], in1=mt[:], op=OP.add)
                nc.sync.dma_start(out=mT[l + 1][:, sl], in_=mn[:])

            out_block(l + 1, mT[l + 1])

        if DBG:
            nc.sync.dma_start(out=dbg["d_xn"][:], in_=xn_row[0:512, :])
            nc.sync.dma_start(out=dbg["d_m0"][:], in_=mT[0][:, 0:512])
            nc.sync.dma_start(out=dbg["d_c"][:], in_=c_shard[0:512, :])
            nc.sync.dma_start(out=dbg["d_xji"][:], in_=xjiT[:, 0:512])
            nc.sync.dma_start(out=dbg["d_agg"][:], in_=aggT[:, 0:512])
            nc.sync.dma_start(out=dbg["d_m1"][:], in_=mT[NLAYERS][:, 0:512])
            nc.sync.dma_start(out=dbg["d_rbf"][:], in_=rbfT_d[:, 0:512])
            dsb = sb.tile([P, 64], F32, tag="dsb", name="dsb")
            nc.vector.tensor_copy(out=dsb[:], in_=sbfp[:, 0:64])
            nc.sync.dma_start(out=dbg["d_sbfp"][:], in_=dsb[:])
        # ---------------- final: hg -> AllReduce -> MLP -------------------
        hgf = sb.tile([P, B], F32, tag="hgf")
        nc.vector.tensor_copy(out=hgf[:], in_=hg_ps[:])
        nc.sync.dma_start(out=hg_d[:], in_=hgf[:])
        if DBG:
            nc.sync.dma_start(out=dbg["d_hg"][:], in_=hgf[:])
        nc.gpsimd.collective_compute(
            "AllReduce", OP.add, replica_groups=[list(range(NC))],
            ins=[hg_d[:]], outs=[hg_r[:]])
        hgs = sb.tile([P, B], F32, tag="hgs")
        nc.sync.dma_start(out=hgs[:], in_=hg_r[:])
        hgb = sb.tile([P, B], BF, tag="hgb")
        nc.vector.tensor_copy(out=hgb[:], in_=hgs[:])

        cdb = sb.tile([11, B], BF, tag="cdb")
        nc.vector.tensor_copy(out=cdb[:], in_=wt["condT"][:])
        pc = pst.tile([64, B], F32, space="PSUM", tag="t1")
        nc.tensor.matmul(out=pc[:], lhsT=wt["c0_w"][:], rhs=cdb[:], start=True, stop=True)
        ch = sb.tile([64, B], BF, tag="ch")
        nc.scalar.activation(out=ch[:], in_=pc[:], func=AF.Relu,
                             bias=wt["c0_b"][:, :1], scale=1.0)
        pc2 = pst.tile([32, B], F32, space="PSUM", tag="t1")
        nc.tensor.matmul(out=pc2[:], lhsT=wt["c1_w"][:], rhs=ch[:], start=True, stop=True)
        hcT = sb.tile([32, B], BF, tag="hcT")
        nc.vector.tensor_scalar_add(out=hcT[:], in0=pc2[:], scalar1=wt["c1_b"][:, :1])

        o1 = []
        for mh in range(2):
            pm_ = pst.tile([P, B], F32, space="PSUM", tag="t1")
            nc.tensor.matmul(out=pm_[:], lhsT=wt["m0a_w"][:, mh * P:(mh + 1) * P],
                             rhs=hgb[:], start=True, stop=False)
            nc.tensor.matmul(out=pm_[:], lhsT=wt["m0b_w"][:, mh * P:(mh + 1) * P],
                             rhs=hcT[:], start=False, stop=True)
            o = sb.tile([P, B], BF, tag=f"o1_{mh}", name=f"o1_{mh}")
            nc.scalar.activation(out=o[:], in_=pm_[:], func=AF.Relu,
                                 bias=wt["m0_b"][:, mh:mh + 1], scale=1.0)
            o1.append(o)
        pm2 = pst.tile([P, B], F32, space="PSUM", tag="t1")
        nc.tensor.matmul(out=pm2[:], lhsT=wt["m1a_w"][:], rhs=o1[0][:], start=True, stop=False)
        nc.tensor.matmul(out=pm2[:], lhsT=wt["m1b_w"][:], rhs=o1[1][:], start=False, stop=True)
        o2 = sb.tile([P, B], BF, tag="o2")
        nc.scalar.activation(out=o2[:], in_=pm2[:], func=AF.Relu,
                             bias=wt["m1_b"][:, :1], scale=1.0)
        pf = pst.tile([1, B], F32, space="PSUM", tag="t1")
        nc.tensor.matmul(out=pf[:], lhsT=wt["m2_w"][:], rhs=o2[:], start=True, stop=True)
        fo = sb.tile([1, B], F32, tag="fo")
        nc.vector.tensor_scalar_add(out=fo[:], in0=pf[:], scalar1=m2_b)
        nc.sync.dma_start(out=out_ap[:], in_=fo[:])
        ctx.close()

    nc.compile()
    return nc


def kernel(**inputs):
    in_maps, m2_b = _build_host(**inputs)
    if "nc" not in _cache:
        _cache["nc"] = _build_program(in_maps[0], m2_b)
    nc = _cache["nc"]
    res = run_bass_kernel_spmd(nc, in_maps, list(range(NC)))
    if os.environ.get("KDBG") == "1":
        _cache["dbg"] = res.results
        _cache["in_maps"] = in_maps
    out = np.asarray(res.results[0]["out"], np.float32)
    return out.reshape(B, 1)


# revision 20
# speedup vs baseline: 1.3256x; 1.3256x over previous
"""DimeNet-style GNN forward on 8 Trainium2 NeuronCores (Bass/Tile).

Sharding: edges sorted by destination node j and split contiguously across 8
cores (node-aligned boundaries); triplets live on the core owning their
destination edge ji, sorted by ji so the triplet->edge scatter-add becomes
windowed one-hot matmuls accumulated in PSUM. Per layer each core computes
c = swish(lin_kj(m)) * (rbf @ W_rbf) on its shard; shards are AllGathered
(bf16) and each core indirect-DMA-gathers the rows its triplets reference
(the m[kj] message gather). Edge->graph readout skips the node tensor: edges
scatter straight into one persistent [HID, B] PSUM accumulator via batch
one-hot masks. Activations are kept transposed [HID=128 partitions, rows] in
bf16 with fp32 PSUM accumulation; small weights are replicated.
"""
import sys, os
for _p in ("/opt/trn_rl_repo", "/root/.axon_site/_ro/trn_rl_repo"):
    if os.path.isdir(_p) and _p not in sys.path:
        sys.path.insert(0, _p)

import math
import numpy as np
import jax.numpy as jnp

import concourse.bacc as bacc
import concourse.tile as tile
from concourse.tile import add_dep_helper
import concourse.mybir as mybir
from concourse.bass import IndirectOffsetOnAxis
from concourse.bass_utils import run_bass_kernel_spmd

DT = mybir.dt
OP = mybir.AluOpType
AF = mybir.ActivationFunctionType
AX = mybir.AxisListType

NC, P, HID, NRBF, NSBF, NBIL = 8, 128, 128, 16, 7, 8
NLAYERS, OUTBLK, B, CUTOFF = 4, 3, 128, 5.0
N, E, T = 50000, 200000, 200000
NP_ = 50048            # padded nodes (391*128)
EP = 25088             # padded per-core edges (196*128)
NWIN = EP // P         # 196
KT = 2                 # triplet tiles per window (fixed -> SPMD-safe)
TTILES = NWIN * KT     # 392
TSLOT = TTILES * P
EA_PAD = 200064
CFULL = NC * EP
CT = EP // 512         # 49

_cache = {}


def _bf16(a):
    return np.asarray(jnp.asarray(np.asarray(a, np.float32), dtype=jnp.bfloat16))


# --------------------------------------------------------------------------
# host-side preprocessing (index math + weight layout only)
# --------------------------------------------------------------------------
def _build_host(x, edge_index, edge_attr, batch, triplet_index, ph_enc,
                temp_enc, rec_flag, box_idx, params):
    g = np.asarray
    i_o = g(edge_index[0]).astype(np.int64)
    j_o = g(edge_index[1]).astype(np.int64)
    kj_o = g(triplet_index[0]).astype(np.int64)
    ji_o = g(triplet_index[1]).astype(np.int64)
    batch_np = g(batch).astype(np.int64)
    x_np = g(x).astype(np.float32)
    ea_np = g(edge_attr).astype(np.float32)

    order_e = np.argsort(j_o, kind="stable")
    j_s, i_s = j_o[order_e], i_o[order_e]
    inv_e = np.empty(E, np.int64)
    inv_e[order_e] = np.arange(E)

    e_bounds = [0]
    for p in range(1, NC):
        t = p * (E // NC)
        while t < E and j_s[t] == j_s[t - 1]:
            t += 1
        e_bounds.append(t)
    e_bounds.append(E)
    e_bounds = np.asarray(e_bounds, np.int64)
    assert (np.diff(e_bounds) <= EP).all()

    ji_n, kj_n = inv_e[ji_o], inv_e[kj_o]
    t_core = np.searchsorted(e_bounds[1:], ji_n, side="right")
    kj_core = np.searchsorted(e_bounds[1:], kj_n, side="right")
    kj_pad = (kj_core * EP + (kj_n - e_bounds[kj_core])).astype(np.int64)

    x_pad = np.zeros((NP_, 64), np.float32)
    x_pad[:N] = x_np
    ea_orig = np.zeros((EA_PAD, NRBF), np.float32)
    ea_orig[:E] = ea_np

    pr = params
    W = {}

    def lin(p_, name, bias=True):
        W[name + "_w"] = _bf16(g(p_["w"]))
        if bias:
            W[name + "_b"] = g(p_["b"]).astype(np.float32).reshape(-1, 1)

    lin(pr["emb"]["node"], "node")
    wl = g(pr["emb"]["lin"]["w"]).astype(np.float32)
    bl = g(pr["emb"]["lin"]["b"]).astype(np.float32)
    we = g(pr["emb"]["edge"]["w"]).astype(np.float32)
    be = g(pr["emb"]["edge"]["b"]).astype(np.float32)
    wr_ = g(pr["emb"]["rbf"]["w"]).astype(np.float32)
    br = g(pr["emb"]["rbf"]["b"]).astype(np.float32)
    W["emb1_w"] = _bf16(wl[0:128])
    W["embe_w"] = _bf16(we @ wl[128:256])
    W["embr_w"] = _bf16(wr_ @ wl[256:384])
    W["emb_b"] = (bl + be @ wl[128:256] + br @ wl[256:384]).reshape(-1, 1).astype(np.float32)

    sbf_all = []
    for li in range(NLAYERS):
        pl = pr["int"][li]
        lin(pl["lin_ji"], f"ji{li}")
        lin(pl["lin_kj"], f"kj{li}")
        W[f"rbf{li}_w"] = _bf16(g(pl["lin_rbf"]["w"]))
        sbf_all.append(g(pl["lin_sbf"]["w"]).astype(np.float32))
        W[f"W{li}"] = _bf16(g(pl["W"]).reshape(HID, NBIL * HID))
        for rn in ("res1", "res2"):
            lin(pl[rn]["lin1"], f"{rn}a{li}")
            lin(pl[rn]["lin2"], f"{rn}b{li}")
        lin(pl["lin_out"], f"lo{li}")
    W["sbf_all_w"] = _bf16(np.concatenate(sbf_all, axis=1))       # [7, 32]
    for oi in range(NLAYERS + 1):
        po = pr["out"][oi]
        for ri in range(OUTBLK):
            lin(po["res"][ri]["lin1"], f"ob{oi}r{ri}a")
            lin(po["res"][ri]["lin2"], f"ob{oi}r{ri}b")
        lin(po["lin"], f"ob{oi}l")
        W[f"ob{oi}l_brow"] = _bf16(g(po["lin"]["b"]).reshape(1, -1))

    c0w = g(pr["cond"][0]["w"]).astype(np.float32)
    W["c0_w"] = _bf16(c0w)                                         # [11, 64]
    W["c0_b"] = g(pr["cond"][0]["b"]).astype(np.float32).reshape(-1, 1)
    W["c1_w"] = _bf16(g(pr["cond"][1]["w"]))                       # [64, 32]
    W["c1_b"] = g(pr["cond"][1]["b"]).astype(np.float32).reshape(-1, 1)
    m0w = g(pr["mlp"][0]["w"]).astype(np.float32)                  # [160, 256]
    W["m0a_w"] = _bf16(m0w[0:128])
    W["m0b_w"] = _bf16(m0w[128:160])
    W["m0_b"] = g(pr["mlp"][0]["b"]).astype(np.float32).reshape(2, 128).T.copy()
    m1w = g(pr["mlp"][1]["w"]).astype(np.float32)                  # [256, 128]
    W["m1a_w"] = _bf16(m1w[0:128])
    W["m1b_w"] = _bf16(m1w[128:256])
    W["m1_b"] = g(pr["mlp"][1]["b"]).astype(np.float32).reshape(-1, 1)
    W["m2_w"] = _bf16(g(pr["mlp"][2]["w"]))                        # [128, 1]
    m2_b = float(g(pr["mlp"][2]["b"]).reshape(-1)[0])

    cont = np.stack([g(ph_enc), g(temp_enc), g(rec_flag)], -1).astype(np.float32)
    cond = np.concatenate([cont, g(pr["box_embed"])[g(box_idx).astype(np.int64)]], -1)
    W["condT"] = cond.T.copy().astype(np.float32)                  # [11, 128]

    consts = {
        "identity": np.eye(P, dtype=np.float32),
        "ident_bf": _bf16(np.eye(P)),
        "iota_bf": _bf16(np.tile(np.arange(P, dtype=np.float32)[None], (P, 1))),
        "freqs": np.tile((np.arange(1, NRBF + 1, dtype=np.float32)
                          * (math.pi / CUTOFF) / (2.0 * math.pi))[None], (P, 1)),
        "ones_bf": _bf16(np.ones((1, P))),
        "x_row": _bf16(x_pad),
        "ea_orig": ea_orig,
    }

    in_maps = []
    for p in range(NC):
        lo, hi = int(e_bounds[p]), int(e_bounds[p + 1])
        ne = hi - lo
        m = dict(W)
        m.update(consts)
        ii = np.zeros(EP, np.int32); ii[:ne] = i_s[lo:hi]
        jj = np.zeros(EP, np.int32); jj[:ne] = j_s[lo:hi]
        bj = np.full(EP, -1.0, np.float32)
        bj[:ne] = batch_np[j_s[lo:hi]].astype(np.float32)
        eaT = np.zeros((NRBF, EP), np.float32)
        eaT[:, :ne] = ea_np[order_e[lo:hi]].T
        ea_row = np.zeros((EP, NRBF), np.float32)
        ea_row[:ne] = ea_np[order_e[lo:hi]]
        m["i_idx"] = ii.reshape(NWIN, P).T.copy()
        m["j_idx"] = jj.reshape(NWIN, P).T.copy()
        m["bj_col"] = bj.reshape(NWIN, P).T.copy()
        m["eaT"] = eaT
        m["ea_row"] = ea_row

        sel = np.where(t_core == p)[0]
        jil = (ji_n[sel] - lo).astype(np.int64)
        o2 = np.argsort(jil, kind="stable")
        sel, jil = sel[o2], jil[o2]
        win = jil // P
        kj_slot = np.zeros(TSLOT, np.int32)
        ko_slot = np.zeros(TSLOT, np.int32)
        jo_slot = np.zeros(TSLOT, np.int32)
        jirel = np.full(TSLOT, -1.0, np.float32)
        st = np.searchsorted(win, np.arange(NWIN))
        en = np.searchsorted(win, np.arange(NWIN), side="right")
        for w in range(NWIN):
            s, e_ = int(st[w]), int(en[w])
            cnt = e_ - s
            assert cnt <= KT * P, f"window overflow {cnt}"
            base = w * KT * P
            kj_slot[base:base + cnt] = kj_pad[sel[s:e_]]
            ko_slot[base:base + cnt] = kj_o[sel[s:e_]]
            jo_slot[base:base + cnt] = ji_o[sel[s:e_]]
            jirel[base:base + cnt] = (jil[s:e_] - w * P).astype(np.float32)
        m["kj_idx"] = kj_slot.reshape(TTILES, P).T.copy()
        m["ko_idx"] = ko_slot.reshape(TTILES, P).T.copy()
        m["jo_idx"] = jo_slot.reshape(TTILES, P).T.copy()
        m["jirel"] = jirel.reshape(TTILES, P).T.copy()
        in_maps.append(m)
    return in_maps, m2_b


# --------------------------------------------------------------------------
# device program
# --------------------------------------------------------------------------
def _build_program(map0, m2_b):
    nc = bacc.Bacc("TRN2", target_bir_lowering=False, debug=False, num_devices=NC)
    aps = {}
    for name, arr in map0.items():
        if arr.dtype == np.float32:
            dt = DT.float32
        elif arr.dtype == np.int32:
            dt = DT.int32
        else:
            dt = DT.bfloat16
        aps[name] = nc.dram_tensor(name, list(arr.shape), dt, kind="ExternalInput").ap()
    out_ap = nc.dram_tensor("out", [1, B], DT.float32, kind="ExternalOutput").ap()
    DBG = os.environ.get("KDBG") == "1"
    dbg = {}
    if DBG:
        for nm, shp, dt_ in [("d_xn", [512, HID], DT.bfloat16),
                             ("d_m0", [P, 512], DT.bfloat16),
                             ("d_c", [512, HID], DT.bfloat16),
                             ("d_xji", [P, 512], DT.bfloat16),
                             ("d_agg", [P, 512], DT.bfloat16),
                             ("d_m1", [P, 512], DT.bfloat16),
                             ("d_hg", [P, B], DT.float32),
                             ("d_sbfp", [P, 64], DT.float32),
                             ("d_rbf", [NRBF, 512], DT.bfloat16)]:
            dbg[nm] = nc.dram_tensor(nm, shp, dt_, kind="ExternalOutput").ap()

    BF, F32 = DT.bfloat16, DT.float32
    with tile.TileContext(nc) as tc:
        from contextlib import ExitStack
        ctx = ExitStack()
        sb = ctx.enter_context(tc.tile_pool(name="sb", bufs=3))
        sbw = ctx.enter_context(tc.tile_pool(name="sbw", bufs=1))
        ps = ctx.enter_context(tc.tile_pool(name="ps", bufs=2, space="PSUM"))
        pst = ctx.enter_context(tc.tile_pool(name="pst", bufs=2, space="PSUM"))
        psG = ctx.enter_context(tc.tile_pool(name="psG", bufs=1, space="PSUM"))
        psh = ctx.enter_context(tc.tile_pool(name="psh", bufs=1, space="PSUM"))
        dr = ctx.enter_context(tc.tile_pool(name="dr", bufs=1, space="DRAM"))

        wt = {}
        for name, a in aps.items():
            if name in ("x_row", "eaT", "ea_row", "ea_orig", "i_idx", "j_idx",
                        "kj_idx", "ko_idx", "jo_idx"):
                continue
            t = sbw.tile(list(a.shape), a.dtype, tag=name, name="w_" + name)
            nc.sync.dma_start(out=t[:], in_=a[:])
            wt[name] = t
        ident, ident_bf = wt["identity"], wt["ident_bf"]
        iota, freqs = wt["iota_bf"], wt["freqs"]

        idx_sb = {}
        for name in ("i_idx", "j_idx", "kj_idx", "ko_idx", "jo_idx"):
            a = aps[name]
            t = sbw.tile(list(a.shape), a.dtype, tag=name, name="i_" + name)
            nc.sync.dma_start(out=t[:], in_=a[:])
            idx_sb[name] = t

        rbfT_d = dr.tile([NRBF, EP], BF, tag="rbfT_d", name="rbfT_d")
        sbfp = sbw.tile([P, TTILES * 32], F32, tag="sbfp")

        # DRAM scratch
        mT = [dr.tile([P, EP], BF, tag=f"mT{l}", name=f"mT{l}") for l in range(NLAYERS + 1)]
        xjiT = dr.tile([P, EP], BF, tag="xjiT")
        aggT = dr.tile([P, EP], BF, tag="aggT")
        c_shard = dr.tile([EP, HID], BF, tag="c_shard")
        c_full = dr.tile([CFULL, HID], BF, tag="c_full")
        hg_d = dr.tile([P, B], F32, tag="hg_d")
        hg_r = dr.tile([P, B], F32, tag="hg_r")

        hg_ps = psh.tile([P, B], F32, space="PSUM")
        hg_n = [0]
        HG_TOTAL = (NLAYERS + 1) * NWIN

        def gather(table, idx_col, width, dtype, after=None):
            tl = sb.tile([P, width], dtype, tag="gather", name="gtile")
            gi = nc.gpsimd.indirect_dma_start(
                out=tl[:], out_offset=None, in_=table,
                in_offset=IndirectOffsetOnAxis(ap=idx_col, axis=0))
            if after is not None:
                add_dep_helper(gi.ins, after, True, "indirect gather after table write")
            return tl

        def silu_evict(psum, bias, tag):
            o = sb.tile([P, psum.shape[-1]], BF, tag=tag, name="se_" + tag)
            nc.scalar.activation(out=o[:], in_=psum[:], func=AF.Silu,
                                 bias=bias, scale=1.0)
            return o

        def residual(xT, wa, ba, wb, bb, width, pref):
            s0 = sb.tile([P, width], BF, tag=pref + "s0", name=pref + "s0")
            nc.scalar.activation(out=s0[:], in_=xT[:], func=AF.Silu, scale=1.0)
            p1 = ps.tile([P, width], F32, space="PSUM", tag="pA", name="p1")
            nc.tensor.matmul(out=p1[:], lhsT=wt[wa][:], rhs=s0[:], start=True, stop=True)
            s1 = silu_evict(p1, wt[ba][:, :1], pref + "s1")
            p2 = ps.tile([P, width], F32, space="PSUM", tag="pA", name="p2")
            nc.tensor.matmul(out=p2[:], lhsT=wt[wb][:], rhs=s1[:], start=True, stop=True)
            u = sb.tile([P, width], BF, tag=pref + "u", name=pref + "u")
            nc.vector.tensor_scalar_add(out=u[:], in0=p2[:], scalar1=wt[bb][:, :1])
            r = sb.tile([P, width], BF, tag=pref + "r", name=pref + "r")
            nc.vector.tensor_tensor(out=r[:], in0=u[:], in1=xT[:], op=OP.add)
            return r

        # ---------------- Phase C0: rbf ----------------------------------
        for t in range(NWIN):
            ea = sb.tile([P, NRBF], F32, tag="ea")
            nc.sync.dma_start(out=ea[:], in_=aps["ea_row"][t * P:(t + 1) * P, :])
            sq = sb.tile([P, NRBF], F32, tag="sq")
            nc.vector.tensor_tensor(out=sq[:], in0=ea[:], in1=ea[:], op=OP.mult)
            d2 = sb.tile([P, 1], F32, tag="d2")
            nc.vector.reduce_sum(out=d2[:], in_=sq[:], axis=AX.X)
            d = sb.tile([P, 1], F32, tag="d")
            nc.scalar.activation(out=d[:], in_=d2[:], func=AF.Sqrt, scale=1.0)
            dp = sb.tile([P, 1], F32, tag="dp")
            nc.vector.tensor_scalar_add(out=dp[:], in0=d[:], scalar1=1e-8)
            rec = sb.tile([P, 1], F32, tag="rec")
            nc.vector.reciprocal(out=rec[:], in_=dp[:])
            # q = f*d/(2pi); y = q - round(q); sin(2pi*y) == sin(f*d)
            q_ = sb.tile([P, NRBF], F32, tag="q_")
            nc.vector.tensor_scalar_mul(out=q_[:], in0=freqs[:], scalar1=d[:, :1])
            qi = sb.tile([P, NRBF], DT.int32, tag="qi")
            nc.vector.tensor_copy(out=qi[:], in_=q_[:])
            qf = sb.tile([P, NRBF], F32, tag="qf")
            nc.vector.tensor_copy(out=qf[:], in_=qi[:])
            yr = sb.tile([P, NRBF], F32, tag="yr")
            nc.vector.tensor_tensor(out=yr[:], in0=q_[:], in1=qf[:], op=OP.subtract)
            sins = sb.tile([P, NRBF], F32, tag="sins")
            nc.scalar.activation(out=sins[:], in_=yr[:], func=AF.Sin,
                                 scale=2.0 * math.pi)
            rrow = sb.tile([P, NRBF], F32, tag="rrow")
            nc.vector.tensor_scalar_mul(out=rrow[:], in0=sins[:], scalar1=rec[:, :1])
            tp = pst.tile([NRBF, P], F32, space="PSUM", tag="t1")
            nc.tensor.transpose(out=tp[:], in_=rrow[:], identity=ident[:])
            rbs = sb.tile([NRBF, P], BF, tag="rbs", name="rbs")
            nc.vector.tensor_copy(out=rbs[:], in_=tp[:])
            nc.sync.dma_start(out=rbfT_d[:, t * P:(t + 1) * P], in_=rbs[:])

        # ---------------- Phase C2: sbf / sbf_proj ------------------------
        for t in range(TTILES):
            ekj = gather(aps["ea_orig"][:], idx_sb["ko_idx"][:, t:t + 1], NRBF, F32)
            eji = gather(aps["ea_orig"][:], idx_sb["jo_idx"][:, t:t + 1], NRBF, F32)
            ab_ = sb.tile([P, NRBF], F32, tag="ab_")
            nc.vector.tensor_tensor(out=ab_[:], in0=ekj[:], in1=eji[:], op=OP.mult)
            ab = sb.tile([P, 1], F32, tag="ab")
            nc.vector.reduce_sum(out=ab[:], in_=ab_[:], axis=AX.X)
            aa_ = sb.tile([P, NRBF], F32, tag="aa_")
            nc.vector.tensor_tensor(out=aa_[:], in0=ekj[:], in1=ekj[:], op=OP.mult)
            aa = sb.tile([P, 1], F32, tag="aa")
            nc.vector.reduce_sum(out=aa[:], in_=aa_[:], axis=AX.X)
            bb_ = sb.tile([P, NRBF], F32, tag="bb_")
            nc.vector.tensor_tensor(out=bb_[:], in0=eji[:], in1=eji[:], op=OP.mult)
            bb = sb.tile([P, 1], F32, tag="bb")
            nc.vector.reduce_sum(out=bb[:], in_=bb_[:], axis=AX.X)
            q = sb.tile([P, 1], F32, tag="q")
            nc.vector.tensor_tensor(out=q[:], in0=aa[:], in1=bb[:], op=OP.mult)
            sq_ = sb.tile([P, 1], F32, tag="sq_")
            nc.scalar.activation(out=sq_[:], in_=q[:], func=AF.Sqrt, scale=1.0)
            sqc = sb.tile([P, 1], F32, tag="sqc")
            nc.vector.tensor_scalar_max(out=sqc[:], in0=sq_[:], scalar1=1e-16)
            rc = sb.tile([P, 1], F32, tag="rc")
            nc.vector.reciprocal(out=rc[:], in_=sqc[:])
            cs = sb.tile([P, 1], F32, tag="cs")
            nc.vector.tensor_scalar(out=cs[:], in0=ab[:], scalar1=rc[:, :1],
                                    scalar2=0.999, op0=OP.mult, op1=OP.min)
            cosv = sb.tile([P, 1], F32, tag="cosv")
            nc.vector.tensor_scalar_max(out=cosv[:], in0=cs[:], scalar1=-0.999)
            c2 = sb.tile([P, 1], F32, tag="c2")
            nc.vector.tensor_tensor(out=c2[:], in0=cosv[:], in1=cosv[:], op=OP.mult)
            sbf_t = sb.tile([P, NSBF], F32, tag="sbf_t")
            nc.scalar.activation(out=sbf_t[:, 0:1], in_=c2[:], func=AF.Sqrt,
                                 scale=-1.0, bias=1.0)
            twoc = sb.tile([P, 1], F32, tag="twoc")
            nc.vector.tensor_scalar_mul(out=twoc[:], in0=cosv[:], scalar1=2.0)
            nc.vector.tensor_scalar(out=sbf_t[:, 1:2], in0=sbf_t[:, 0:1],
                                    scalar1=twoc[:, :1], scalar2=None, op0=OP.mult)
            for k in range(2, NSBF):
                tmp = sb.tile([P, 1], F32, tag="sbtmp")
                nc.vector.tensor_scalar(out=tmp[:], in0=sbf_t[:, k - 1:k],
                                        scalar1=twoc[:, :1], scalar2=None, op0=OP.mult)
                nc.vector.tensor_tensor(out=sbf_t[:, k:k + 1], in0=tmp[:],
                                        in1=sbf_t[:, k - 2:k - 1], op=OP.subtract)
            tp7 = pst.tile([NSBF, P], F32, space="PSUM", tag="t1")
            nc.tensor.transpose(out=tp7[:], in_=sbf_t[:], identity=ident[:])
            sbfTt = sb.tile([NSBF, P], BF, tag="sbfTt")
            nc.vector.tensor_copy(out=sbfTt[:], in_=tp7[:])
            pj = pst.tile([P, 32], F32, space="PSUM", tag="t1")
            nc.tensor.matmul(out=pj[:], lhsT=sbfTt[:], rhs=wt["sbf_all_w"][:],
                             start=True, stop=True)
            nc.vector.tensor_copy(out=sbfp[:, t * 32:(t + 1) * 32], in_=pj[:])

        # ---------------- Phase C: embedding -> mT[0] ---------------------
        for t in range(NWIN):
            xi = gather(aps["x_row"][:], idx_sb["i_idx"][:, t:t + 1], 64, BF)
            xj = gather(aps["x_row"][:], idx_sb["j_idx"][:, t:t + 1], 64, BF)
            xiT_ps = pst.tile([64, P], BF, space="PSUM", tag="t1", name="xiT_ps")
            nc.tensor.transpose(out=xiT_ps[:], in_=xi[:], identity=ident_bf[:])
            xiT = sb.tile([64, P], BF, tag="xiT")
            nc.vector.tensor_copy(out=xiT[:], in_=xiT_ps[:])
            xjT_ps = pst.tile([64, P], BF, space="PSUM", tag="t1", name="xjT_ps")
            nc.tensor.transpose(out=xjT_ps[:], in_=xj[:], identity=ident_bf[:])
            xjT = sb.tile([64, P], BF, tag="xjT")
            nc.vector.tensor_copy(out=xjT[:], in_=xjT_ps[:])
            pxi = pst.tile([P, P], F32, space="PSUM", tag="t1", name="pxi")
            nc.tensor.matmul(out=pxi[:], lhsT=wt["node_w"][:], rhs=xiT[:],
                             start=True, stop=True)
            xne = sb.tile([P, P], BF, tag="xne")
            nc.vector.tensor_scalar_add(out=xne[:], in0=pxi[:],
                                        scalar1=wt["node_b"][:, :1])
            pxj = pst.tile([P, P], F32, space="PSUM", tag="t1", name="pxj")
            nc.tensor.matmul(out=pxj[:], lhsT=wt["node_w"][:], rhs=xjT[:],
                             start=True, stop=True)
            xnj = sb.tile([P, P], BF, tag="xnj")
            nc.vector.tensor_scalar_add(out=xnj[:], in0=pxj[:],
                                        scalar1=wt["node_b"][:, :1])
            xijT = sb.tile([P, P], BF, tag="xijT")
            nc.vector.tensor_tensor(out=xijT[:], in0=xne[:], in1=xnj[:], op=OP.mult)
            ea = sb.tile([NRBF, P], F32, tag="eaTl")
            nc.sync.dma_start(out=ea[:], in_=aps["eaT"][:, t * P:(t + 1) * P])
            eab = sb.tile([NRBF, P], BF, tag="eaTb")
            nc.vector.tensor_copy(out=eab[:], in_=ea[:])
            pm = ps.tile([P, P], F32, space="PSUM", tag="pA")
            nc.tensor.matmul(out=pm[:], lhsT=wt["emb1_w"][:], rhs=xijT[:],
                             start=True, stop=False)
            nc.tensor.matmul(out=pm[:], lhsT=wt["embe_w"][:], rhs=eab[:],
                             start=False, stop=False)
            rbl = sb.tile([NRBF, P], BF, tag="rbl", name="rbl")
            nc.sync.dma_start(out=rbl[:], in_=rbfT_d[:, t * P:(t + 1) * P])
            nc.tensor.matmul(out=pm[:], lhsT=wt["embr_w"][:],
                             rhs=rbl[:], start=False, stop=True)
            m0t = silu_evict(pm, wt["emb_b"][:, :1], "m0t")
            nc.sync.dma_start(out=mT[0][:, t * P:(t + 1) * P], in_=m0t[:])

        def hg_scatter(me_row, eb):
            sg = sb.tile([P, B], BF, tag="sg")
            nc.vector.tensor_scalar(out=sg[:], in0=iota[:],
                                    scalar1=wt["bj_col"][:, eb:eb + 1],
                                    scalar2=None, op0=OP.is_equal)
            nc.tensor.matmul(out=hg_ps[:], lhsT=me_row[:], rhs=sg[:],
                             start=(hg_n[0] == 0), stop=(hg_n[0] == HG_TOTAL * 4 - 1),
                             skip_group_check=True)
            hg_n[0] += 1

        def out_block(oi, mcur):
            for c in range(CT):
                mt = sb.tile([P, 512], BF, tag="ob_m")
                nc.sync.dma_start(out=mt[:], in_=mcur[:, c * 512:(c + 1) * 512])
                r = mt
                for ri in range(OUTBLK):
                    r = residual(r, f"ob{oi}r{ri}a_w", f"ob{oi}r{ri}a_b",
                                 f"ob{oi}r{ri}b_w", f"ob{oi}r{ri}b_b", 512, "ob")
                for k in range(4):
                    pr_ = pst.tile([P, P], F32, space="PSUM", tag="t1")
                    nc.tensor.matmul(out=pr_[:], lhsT=r[:, k * P:(k + 1) * P],
                                     rhs=wt[f"ob{oi}l_w"][:], start=True, stop=False)
                    nc.tensor.matmul(out=pr_[:], lhsT=wt["ones_bf"][:1, :],
                                     rhs=wt[f"ob{oi}l_brow"][:1, :],
                                     start=False, stop=True)
                    me = sb.tile([P, P], BF, tag="ob_me")
                    nc.scalar.activation(out=me[:], in_=pr_[:], func=AF.Silu, scale=1.0)
                    hg_scatter(me, c * 4 + k)

        out_block(0, mT[0])

        # ---------------- layers ------------------------------------------
        for l in range(NLAYERS):
            # P1: x_ji, c
            for c in range(CT):
                sl = slice(c * 512, (c + 1) * 512)
                mt = sb.tile([P, 512], BF, tag="p1_m")
                nc.sync.dma_start(out=mt[:], in_=mT[l][:, sl])
                p1 = ps.tile([P, 512], F32, space="PSUM", tag="pA")
                nc.tensor.matmul(out=p1[:], lhsT=wt[f"ji{l}_w"][:], rhs=mt[:],
                                 start=True, stop=True)
                xji = silu_evict(p1, wt[f"ji{l}_b"][:, :1], "p1_xji")
                nc.sync.dma_start(out=xjiT[:, sl], in_=xji[:])
                p2 = ps.tile([P, 512], F32, space="PSUM", tag="pA")
                nc.tensor.matmul(out=p2[:], lhsT=wt[f"kj{l}_w"][:], rhs=mt[:],
                                 start=True, stop=True)
                t1 = silu_evict(p2, wt[f"kj{l}_b"][:, :1], "p1_t1")
                rbl5 = sb.tile([NRBF, 512], BF, tag="rbl5", name="rbl5")
                nc.sync.dma_start(out=rbl5[:], in_=rbfT_d[:, sl])
                p3 = ps.tile([P, 512], F32, space="PSUM", tag="pA")
                nc.tensor.matmul(out=p3[:], lhsT=wt[f"rbf{l}_w"][:],
                                 rhs=rbl5[:], start=True, stop=True)
                ct_ = sb.tile([P, 512], BF, tag="p1_c")
                nc.vector.tensor_tensor(out=ct_[:], in0=t1[:], in1=p3[:], op=OP.mult)
                stage = sb.tile([P, 512], BF, tag="p1_st")
                for k in range(4):
                    tp = pst.tile([P, P], BF, space="PSUM", tag="t1", name="tp")
                    nc.tensor.transpose(out=tp[:], in_=ct_[:, k * P:(k + 1) * P],
                                        identity=ident_bf[:])
                    nc.vector.tensor_copy(out=stage[:, k * P:(k + 1) * P], in_=tp[:])
                dst = c_shard[c * 512:(c + 1) * 512, :].rearrange(
                    "(k p) f -> p k f", k=4, p=P)
                nc.sync.dma_start(out=dst, in_=stage[:])

            ag_inst = nc.gpsimd.collective_compute(
                "AllGather", OP.bypass, replica_groups=[list(range(NC))],
                ins=[c_shard[:]], outs=[c_full[:]]).ins

            # P3: triplets -> aggT
            for w in range(NWIN):
                G = psG.tile([P, NBIL * P], F32, space="PSUM", tag="G")
                for kt in range(KT):
                    t = w * KT + kt
                    cg = gather(c_full[:], idx_sb["kj_idx"][:, t:t + 1], HID, BF,
                                after=ag_inst)
                    for b in range(NBIL):
                        S = sb.tile([P, P], BF, tag="S")
                        nc.vector.tensor_scalar(
                            out=S[:], in0=iota[:],
                            scalar1=wt["jirel"][:, t:t + 1],
                            scalar2=sbfp[:, t * 32 + l * 8 + b:t * 32 + l * 8 + b + 1],
                            op0=OP.is_equal, op1=OP.mult)
                        nc.tensor.matmul(out=G[:, b * P:(b + 1) * P], lhsT=cg[:],
                                         rhs=S[:], start=(kt == 0), stop=(kt == KT - 1))
                Gs = sb.tile([P, NBIL * P], BF, tag="Gs")
                for b in range(NBIL):
                    slb = slice(b * P, (b + 1) * P)
                    if b % 2 == 0:
                        nc.vector.tensor_copy(out=Gs[:, slb], in_=G[:, slb])
                    else:
                        nc.scalar.activation(out=Gs[:, slb], in_=G[:, slb],
                                             func=AF.Copy, scale=1.0)
                pa = pst.tile([P, P], F32, space="PSUM", tag="t1")
                for b in range(NBIL):
                    slb = slice(b * P, (b + 1) * P)
                    nc.tensor.matmul(out=pa[:], lhsT=wt[f"W{l}"][:, slb],
                                     rhs=Gs[:, slb], start=(b == 0), stop=(b == NBIL - 1))
                av = sb.tile([P, P], BF, tag="av")
                nc.vector.tensor_copy(out=av[:], in_=pa[:])
                nc.sync.dma_start(out=aggT[:, w * P:(w + 1) * P], in_=av[:])

            # P4: residual chain -> mT[l+1]
            for c in range(CT):
                sl = slice(c * 512, (c + 1) * 512)
                mt = sb.tile([P, 512], BF, tag="p4_m")
                nc.sync.dma_start(out=mt[:], in_=mT[l][:, sl])
                xji = sb.tile([P, 512], BF, tag="p4_x")
                nc.sync.dma_start(out=xji[:], in_=xjiT[:, sl])
                ag = sb.tile([P, 512], BF, tag="p4_a")
                nc.sync.dma_start(out=ag[:], in_=aggT[:, sl])
                r0 = sb.tile([P, 512], BF, tag="p4_r0")
                nc.vector.tensor_tensor(out=r0[:], in0=xji[:], in1=ag[:], op=OP.add)
                r1 = residual(r0, f"res1a{l}_w", f"res1a{l}_b",
                              f"res1b{l}_w", f"res1b{l}_b", 512, "p4")
                r2 = residual(r1, f"res2a{l}_w", f"res2a{l}_b",
                              f"res2b{l}_w", f"res2b{l}_b", 512, "p4")
                po = ps.tile([P, 512], F32, space="PSUM", tag="pA")
                nc.tensor.matmul(out=po[:], lhsT=wt[f"lo{l}_w"][:], rhs=r2[:],
                                 start=True, stop=True)
                sw = silu_evict(po, wt[f"lo{l}_b"][:, :1], "p4_sw")
                mn = sb.tile([P, 512], BF, tag="p4_mn")
                nc.vector.tensor_tensor(out=mn[:], in0=sw[:], in1=mt[:], op=OP.add)
                nc.sync.dma_start(out=mT[l + 1][:, sl], in_=mn[:])

            out_block(l + 1, mT[l + 1])

        if DBG:
            nc.sync.dma_start(out=dbg["d_m0"][:], in_=mT[0][:, 0:512])
            nc.sync.dma_start(out=dbg["d_c"][:], in_=c_shard[0:512, :])
            nc.sync.dma_start(out=dbg["d_xji"][:], in_=xjiT[:, 0:512])
            nc.sync.dma_start(out=dbg["d_agg"][:], in_=aggT[:, 0:512])
            nc.sync.dma_start(out=dbg["d_m1"][:], in_=mT[NLAYERS][:, 0:512])
            nc.sync.dma_start(out=dbg["d_rbf"][:], in_=rbfT_d[:, 0:512])
            dsb = sb.tile([P, 64], F32, tag="dsb", name="dsb")
            nc.vector.tensor_copy(out=dsb[:], in_=sbfp[:, 0:64])
            nc.sync.dma_start(out=dbg["d_sbfp"][:], in_=dsb[:])
        # ---------------- final: hg -> AllReduce -> MLP -------------------
        hgf = sb.tile([P, B], F32, tag="hgf")
        nc.vector.tensor_copy(out=hgf[:], in_=hg_ps[:])
        nc.sync.dma_start(out=hg_d[:], in_=hgf[:])
        if DBG:
            nc.sync.dma_start(out=dbg["d_hg"][:], in_=hgf[:])
        nc.gpsimd.collective_compute(
            "AllReduce", OP.add, replica_groups=[list(range(NC))],
            ins=[hg_d[:]], outs=[hg_r[:]])
        hgs = sb.tile([P, B], F32, tag="hgs")
        nc.sync.dma_start(out=hgs[:], in_=hg_r[:])
        hgb = sb.tile([P, B], BF, tag="hgb")
        nc.vector.tensor_copy(out=hgb[:], in_=hgs[:])

        cdb = sb.tile([11, B], BF, tag="cdb")
        nc.vector.tensor_copy(out=cdb[:], in_=wt["condT"][:])
        pc = pst.tile([64, B], F32, space="PSUM", tag="t1")
        nc.tensor.matmul(out=pc[:], lhsT=wt["c0_w"][:], rhs=cdb[:], start=True, stop=True)
        ch = sb.tile([64, B], BF, tag="ch")
        nc.scalar.activation(out=ch[:], in_=pc[:], func=AF.Relu,
                             bias=wt["c0_b"][:, :1], scale=1.0)
        pc2 = pst.tile([32, B], F32, space="PSUM", tag="t1")
        nc.tensor.matmul(out=pc2[:], lhsT=wt["c1_w"][:], rhs=ch[:], start=True, stop=True)
        hcT = sb.tile([32, B], BF, tag="hcT")
        nc.vector.tensor_scalar_add(out=hcT[:], in0=pc2[:], scalar1=wt["c1_b"][:, :1])

        o1 = []
        for mh in range(2):
            pm_ = pst.tile([P, B], F32, space="PSUM", tag="t1")
            nc.tensor.matmul(out=pm_[:], lhsT=wt["m0a_w"][:, mh * P:(mh + 1) * P],
                             rhs=hgb[:], start=True, stop=False)
            nc.tensor.matmul(out=pm_[:], lhsT=wt["m0b_w"][:, mh * P:(mh + 1) * P],
                             rhs=hcT[:], start=False, stop=True)
            o = sb.tile([P, B], BF, tag=f"o1_{mh}", name=f"o1_{mh}")
            nc.scalar.activation(out=o[:], in_=pm_[:], func=AF.Relu,
                                 bias=wt["m0_b"][:, mh:mh + 1], scale=1.0)
            o1.append(o)
        pm2 = pst.tile([P, B], F32, space="PSUM", tag="t1")
        nc.tensor.matmul(out=pm2[:], lhsT=wt["m1a_w"][:], rhs=o1[0][:], start=True, stop=False)
        nc.tensor.matmul(out=pm2[:], lhsT=wt["m1b_w"][:], rhs=o1[1][:], start=False, stop=True)
        o2 = sb.tile([P, B], BF, tag="o2")
        nc.scalar.activation(out=o2[:], in_=pm2[:], func=AF.Relu,
                             bias=wt["m1_b"][:, :1], scale=1.0)
        pf = pst.tile([1, B], F32, space="PSUM", tag="t1")
        nc.tensor.matmul(out=pf[:], lhsT=wt["m2_w"][:], rhs=o2[:], start=True, stop=True)
        fo = sb.tile([1, B], F32, tag="fo")
        nc.vector.tensor_scalar_add(out=fo[:], in0=pf[:], scalar1=m2_b)
        nc.sync.dma_start(out=out_ap[:], in_=fo[:])
        ctx.close()

    nc.compile()
    return nc


def kernel(**inputs):
    in_maps, m2_b = _build_host(**inputs)
    if "nc" not in _cache:
        _cache["nc"] = _build_program(in_maps[0], m2_b)
    nc = _cache["nc"]
    res = run_bass_kernel_spmd(nc, in_maps, list(range(NC)))
    if os.environ.get("KDBG") == "1":
        _cache["dbg"] = res.results
        _cache["in_maps"] = in_maps
    out = np.asarray(res.results[0]["out"], np.float32)
    return out.reshape(B, 1)
